# revision 3
# baseline (speedup 1.0000x reference)
"""Trainium2 kernel for nn_ASCRM_7619271983683 (sparse_attention).

Strategy: pure data parallelism over batch N=8 across the 8 NeuronCores
(one image per core), exactly as the sharding hint suggests. All ops in
the module are batch-local, so no collectives are needed: each core runs
the full per-image pipeline (unfold -> scrambled patch attention ->
gating -> residual -> two shared-weight depthwise-separable conv branches
-> concat -> final depthwise-separable conv), and the host gathers the
8 per-image outputs into the full [8, 128, 128, 128] result.

Key compute-saving transformation vs. the naive graph: the patch
reconstruction crops nph*K = 217 -> 128, so only patches with
ph <= 18 and pw <= 18 (361 of 961) ever reach the output. We only
compute attention for those patches (2.66x less attention work). The
reshape [N,C,nph,npw,K,K] -> [B, C, D] in the reference is a raw
row-major reinterpretation, which we reproduce exactly by building the
unfolded matrix X = [C*961, 49] per image and slicing 128-row chunks.
"""

import numpy as np
import jax
import jax.numpy as jnp
from jax import lax
from functools import partial

EPS = 1e-5
K, S = 7, 4
N, C, H, W = 8, 128, 128, 128
NPH = (H - K) // S + 1      # 31
NPW = (W - K) // S + 1      # 31
NPATCH = NPH * NPW          # 961
D = K * K                   # 49
PKEEP = 19                  # patches with ph,pw <= 18 survive the crop


def _bn(x, g, b):
    return x * (g / np.sqrt(1.0 + EPS)).reshape(1, -1, 1, 1) + b.reshape(1, -1, 1, 1)


_BF = jnp.bfloat16


def _conv2d(x, w, groups=1, pad=0):
    # bf16 operands, fp32 accumulate: PE runs bf16 matmul at 4x the fp32 rate.
    return lax.conv_general_dilated(
        x.astype(_BF), w.astype(_BF), (1, 1), [(pad, pad), (pad, pad)],
        dimension_numbers=("NCHW", "OIHW", "NCHW"), feature_group_count=groups,
        preferred_element_type=jnp.float32)


def _dsconv(x, dw_w, g1, b1, pw_w, g2, b2):
    c_in = x.shape[1]
    y = jax.nn.relu(_bn(_conv2d(x, dw_w, groups=c_in, pad=1), g1, b1))
    y = jax.nn.relu(_bn(_conv2d(y, pw_w, groups=1, pad=0), g2, b2))
    return y


def _unfold1(x):
    # x: [C, H, W] -> [C, nph, npw, K, K]
    idx_h = jnp.arange(NPH)[:, None] * S + jnp.arange(K)[None, :]
    idx_w = jnp.arange(NPW)[:, None] * S + jnp.arange(K)[None, :]
    p = x[:, idx_h, :]                   # [C, nph, K, W]
    p = p[:, :, :, idx_w]                # [C, nph, K, npw, K]
    return p.transpose(0, 1, 3, 2, 4)    # [C, nph, npw, K, K]


def _per_image(ex, q, keep_p, w_conv_e, w_gate1, w_gate2,
               dw1_w, bn1a_g, bn1a_b, pw1_w, bn1b_g, bn1b_b,
               dwf_w, bnfa_g, bnfa_b, pwf_w, bnfb_g, bnfb_b):
    """ex, q: [C, H, W] single image. keep_p: [361] int32 patch ids."""
    # Unfold, then the *raw reinterpretation* [C,nph,npw,K,K] -> [C*961, 49].
    Xe = _unfold1(ex).reshape(C * NPATCH, D)
    Xq = _unfold1(q).reshape(C * NPATCH, D)

    # Gather only the 361 needed 128-row chunks: rows [128p, 128p+128).
    row0 = keep_p * 128                                   # [361]
    rows = row0[:, None] + jnp.arange(128)[None, :]       # [361, 128]
    E = Xe[rows]                                          # [361, 128, 49]
    Q = Xq[rows]                                          # [361, 128, 49]

    Eb = E.astype(_BF)
    Qb = Q.astype(_BF)
    ex_corr = jnp.einsum("oc,bcd->bod", w_conv_e.astype(_BF), Eb,
                         preferred_element_type=jnp.float32)   # [361, 128, 49]
    A = jnp.einsum("bcd,bce->bde", ex_corr.astype(_BF), Qb,
                   preferred_element_type=jnp.float32)         # [361, 49, 49]
    A1 = jax.nn.softmax(A, axis=1)
    B2 = jax.nn.softmax(A, axis=2)
    q_att = jnp.einsum("bcd,bde->bce", Eb, A1.astype(_BF),
                       preferred_element_type=jnp.float32)     # [361, 128, 49]
    ex_att = jnp.einsum("bce,bde->bcd", Qb, B2.astype(_BF),
                        preferred_element_type=jnp.float32)    # [361, 128, 49]

    def recon(att):
        # att: [361, C, 49] for patches (ph, pw) in [0,19)x[0,19)
        a = att.reshape(PKEEP, PKEEP, C, K, K)
        a = a.transpose(2, 0, 3, 1, 4).reshape(C, PKEEP * K, PKEEP * K)
        return a[:, :H, :W]

    q_att = recon(q_att)[None]                            # [1, C, H, W]
    ex_att = recon(ex_att)[None]

    ex_i = ex[None]
    q_i = q[None]
    ex_mask = jax.nn.sigmoid(jnp.einsum("oc,nchw->nohw", w_gate1, ex_att))
    ex_att = ex_att * ex_mask
    exemplar_out = _dsconv(ex_att + ex_i, dw1_w, bn1a_g, bn1a_b,
                           pw1_w, bn1b_g, bn1b_b)
    q_mask = jax.nn.sigmoid(jnp.einsum("oc,nchw->nohw", w_gate2, q_att))
    q_att = q_att * q_mask
    query_out = _dsconv(q_att + q_i, dw1_w, bn1a_g, bn1a_b,
                        pw1_w, bn1b_g, bn1b_b)

    pred = _dsconv(jnp.concatenate([exemplar_out, query_out], axis=1),
                   dwf_w, bnfa_g, bnfa_b, pwf_w, bnfb_g, bnfb_b)
    return pred[0]                                        # [C, H, W]


_COMPILED = {}


def _get_compiled():
    if "fn" not in _COMPILED:
        devs = jax.devices()[:8]
        fn = jax.pmap(_per_image, devices=devs,
                      in_axes=(0, 0) + (None,) * 16)
        _COMPILED["fn"] = fn
    return _COMPILED["fn"]


def kernel(exemplar, query, w_conv_e, w_gate1, w_gate2,
           dw1_w, bn1a_g, bn1a_b, pw1_w, bn1b_g, bn1b_b,
           dwf_w, bnfa_g, bnfa_b, pwf_w, bnfb_g, bnfb_b):
    # Needed patch ids: p = ph*31 + pw with ph, pw in [0, 19).
    ph = np.arange(PKEEP)
    keep = (ph[:, None] * NPW + np.arange(PKEEP)[None, :]).reshape(-1)
    keep = keep.astype(np.int32)

    fn = _get_compiled()
    out = fn(jnp.asarray(exemplar), jnp.asarray(query), keep,
             jnp.asarray(w_conv_e), jnp.asarray(w_gate1), jnp.asarray(w_gate2),
             jnp.asarray(dw1_w), jnp.asarray(bn1a_g), jnp.asarray(bn1a_b),
             jnp.asarray(pw1_w), jnp.asarray(bn1b_g), jnp.asarray(bn1b_b),
             jnp.asarray(dwf_w), jnp.asarray(bnfa_g), jnp.asarray(bnfa_b),
             jnp.asarray(pwf_w), jnp.asarray(bnfb_g), jnp.asarray(bnfb_b))
    return np.asarray(out).astype(np.float32)
